# revision 6
# baseline (speedup 1.0000x reference)
"""Trainium2 Bass kernel for nn_CausalAttention (GNN message passing).

Math (reference):
    pairs[e] = [img[:, src[e]] ; text[:, tgt[e]]]          # B == H == 128
    a[e]     = sigmoid(w2 . relu(W1 @ pairs[e] + b1) + b2) # per-edge gate
    att_img[b, i] = sum_{e: src[e]=i} a[e] * text[b, tgt[e]]
    att_txt[b, t] = sum_{e: tgt[e]=t} a[e] * img[b, src[e]]

Architecture: output-column sharding, fully on-chip. Core c owns
att_img[:, Wc] and att_txt[:, Wc], Wc = [128c, 128c+128).
All tables are precomputed on host in f32 and shipped f16 (tolerance is
2e-2; f16 end-to-end error is ~1e-3). For the img pipe (txt symmetric):
  - edges with src in Wc, bucketed by w = tgt >> 7 (8 buckets of 5
    blocks of 128 edge slots; unused slots are dummies with -1 keys).
  - phase A (gate): h = relu(UwinT.T @ ohKT + V8[w].T @ ohLT + b1) via
    one-hot gather matmuls (f16, PSUM f32), relu on ACT into h16.
  - w2 dot: 10 matmuls with w2 [128,1] stationary over h16 chunks of
    512 -> z rows [10, 512]; sigmoid on ACT; 4 PE transposes put a into
    slot layout a_slot[p, col], col = 10*(b%4) + b//4 for block b.
  - phase B: per block, ohKa = (iota==loc)*a on DVE/GPSIMD (alternating)
    then M_w[lo, loc] += ohlo_b.T @ ohKa on PE (PSUM accum over 5 blocks).
  - tail: att[:, loc] = sum_w txtT8[w].T @ M_w (8 f16 matmuls).
"""

import sys

for _p in ("/opt/trn_rl_repo", "/root/.axon_site/_ro/trn_rl_repo"):
    if _p not in sys.path:
        sys.path.insert(0, _p)

import numpy as np

import concourse.bass as bass
import concourse.tile as tile
from concourse import bacc, mybir

P = 128
DIM = 1024
E = 32768
NCORES = 8
NW = 8            # hi buckets
BPW = 5           # blocks per bucket (capacity 640 vs mean 512)
NBLK = NW * BPW   # 40
EC = NBLK * P     # 5120 edge slots per pipeline
BW = BPW * P      # 640 edges per bucket
HC = EC // 2      # DMA half size

F32 = mybir.dt.float32
F16 = mybir.dt.float16

IS_EQ = mybir.AluOpType.is_equal
MULT = mybir.AluOpType.mult

# f16 const-pack free-dim layout (per partition)
OFF_WINT = 0                 # [0:128] UwinT (pipe i), [128:256] VwinT (pipe t)
OFF_TAB = 256                # V8 (pipe i) then U8 (pipe t), each 8*128
OFF_FEAT = OFF_TAB + 2048    # txtT8 (pipe i tail) then imgT8 (pipe t tail)
OFF_W2 = OFF_FEAT + 2048     # [.. : ..+1] w2 column
OFF_IOTA = OFF_W2 + 1        # [128, 128] iota along free dim
OFF_ID = OFF_IOTA + P        # [128, 128] identity
F16_FREE = OFF_ID + P        # 4609

# f32 pack: b1 | b2 | loc8_i | loc8_t
P32_B1 = 0
P32_B2 = 1
P32_LOC_I = 2
P32_LOC_T = 2 + NBLK
F32_FREE = 2 + 2 * NBLK


def _build_program():
    nc = bacc.Bacc(None, target_bir_lowering=False, debug=False)

    d16 = nc.dram_tensor("pack16", [P, F16_FREE], F16, kind="ExternalInput")
    d32 = nc.dram_tensor("pack32", [P, F32_FREE], F32, kind="ExternalInput")
    doh = {}
    for s in ("i", "t"):
        for k in ("ohkt", "ohlt", "ohlo"):
            doh[(s, k)] = nc.dram_tensor(
                f"{s}_{k}", [P, EC], F16, kind="ExternalInput")
    out_img = nc.dram_tensor("out_img", [P, P], F32, kind="ExternalOutput")
    out_txt = nc.dram_tensor("out_txt", [P, P], F32, kind="ExternalOutput")

    with tile.TileContext(nc) as tc:
        with (
            tc.tile_pool(name="const", bufs=1) as cp,
            tc.tile_pool(name="work", bufs=2) as wp,
            tc.tile_pool(name="psH", bufs=2, space="PSUM") as psH,
            tc.tile_pool(name="psW", bufs=1, space="PSUM") as psW,
            tc.tile_pool(name="psM", bufs=1, space="PSUM") as psM,
            tc.tile_pool(name="psA", bufs=1, space="PSUM") as psA,
        ):
            pack16 = cp.tile([P, F16_FREE], F16)
            pack32 = cp.tile([P, F32_FREE], F32)
            nc.sync.dma_start(pack16[:], d16[:])
            nc.sync.dma_start(pack32[:], d32[:])

            oh = {}
            for s in ("i", "t"):
                for k in ("ohkt", "ohlt", "ohlo"):
                    oh[(s, k)] = cp.tile([P, EC], F16, tag=f"{s}{k}",
                                         name=f"oh_{s}_{k}")
            # pipe i one-hots: halves on scalar (kt/lt) + vector (lo) queues
            for h in (0, 1):
                sl = slice(h * HC, (h + 1) * HC)
                nc.scalar.dma_start(oh[("i", "ohkt")][:, sl],
                                    doh[("i", "ohkt")][:, sl])
                nc.scalar.dma_start(oh[("i", "ohlt")][:, sl],
                                    doh[("i", "ohlt")][:, sl])
            for h in (0, 1):
                sl = slice(h * HC, (h + 1) * HC)
                nc.sync.dma_start(oh[("i", "ohlo")][:, sl],
                                  doh[("i", "ohlo")][:, sl])
            # pipe t one-hots: gpsimd queue, whole tensors
            for k in ("ohkt", "ohlt", "ohlo"):
                nc.gpsimd.dma_start(oh[("t", k)][:], doh[("t", k)][:])

            b1c = pack32[:, P32_B1:P32_B1 + 1]
            b2c = pack32[:, P32_B2:P32_B2 + 1]
            w2c = pack16[:, OFF_W2:OFF_W2 + 1]
            iota16 = pack16[:, OFF_IOTA:OFF_IOTA + P]
            id16 = pack16[:, OFF_ID:OFF_ID + P]

            for si, (s, out_d) in enumerate((("i", out_img), ("t", out_txt))):
                winT = pack16[:, OFF_WINT + si * P:OFF_WINT + (si + 1) * P]
                loc8 = pack32[:, (P32_LOC_I if s == "i" else P32_LOC_T):
                              (P32_LOC_I if s == "i" else P32_LOC_T) + NBLK]
                ohkt = oh[(s, "ohkt")]
                ohlt = oh[(s, "ohlt")]
                ohlo = oh[(s, "ohlo")]
                h16 = cp.tile([P, EC], F16, tag=f"h16{s}")
                a_slot = cp.tile([P, NBLK], F32, tag=f"a_slot{s}")

                def tab8(w):
                    o = OFF_TAB + si * 1024 + w * P
                    return pack16[:, o:o + P]

                def feat8(w):
                    o = OFF_FEAT + si * 1024 + w * P
                    return pack16[:, o:o + P]

                # ---- phase A: h = relu(winT.T@ohKT + tab8[w].T@ohLT + b1)
                for w in range(NW):
                    e0 = w * BW
                    h_ps = psH.tile([P, BW], F32, tag="h_ps")
                    for o, n in ((0, 512), (512, P)):
                        nc.tensor.matmul(
                            h_ps[:, o:o + n], winT, ohkt[:, e0 + o:e0 + o + n],
                            start=True, stop=False)
                    for o, n in ((0, 512), (512, P)):
                        nc.tensor.matmul(
                            h_ps[:, o:o + n], tab8(w), ohlt[:, e0 + o:e0 + o + n],
                            start=False, stop=True)
                    nc.scalar.activation(
                        h16[:, e0:e0 + BW], h_ps[:],
                        mybir.ActivationFunctionType.Relu, bias=b1c)

                # ---- w2 dot: per-block N=1 matmuls (h16 f16 stationary),
                # lands z directly in slot layout [p, block]
                a_ps = psW.tile([P, NBLK], F32, tag="a_ps")
                for b in range(NBLK):
                    nc.tensor.matmul(
                        a_ps[:, b:b + 1], h16[:, b * P:(b + 1) * P], w2c,
                        start=True, stop=True, skip_group_check=True)
                nc.scalar.activation(
                    a_slot[:], a_ps[:],
                    mybir.ActivationFunctionType.Sigmoid, bias=b2c)

                # ---- phase B: M_w[lo, loc] += ohlo_b.T @ ((iota==loc)*a)
                m_ps0 = psM.tile([P, 4 * P], F32, tag="m0")
                m_ps1 = psM.tile([P, 4 * P], F32, tag="m1")
                m_ps = [m_ps0, m_ps1]
                for b in range(NBLK):
                    w, j = b // BPW, b % BPW
                    oh_a = wp.tile([P, P], F16, tag=f"ohKa{b % 2}")
                    eng = nc.vector if b % 2 == 0 else nc.gpsimd
                    eng.tensor_scalar(
                        out=oh_a[:], in0=iota16,
                        scalar1=loc8[:, b:b + 1], scalar2=a_slot[:, b:b + 1],
                        op0=IS_EQ, op1=MULT)
                    mslice = m_ps[w // 4][:, (w % 4) * P:(w % 4 + 1) * P]
                    nc.tensor.matmul(
                        mslice, ohlo[:, b * P:(b + 1) * P], oh_a[:],
                        start=(j == 0), stop=(j == BPW - 1),
                        skip_group_check=True)

                # ---- tail: att[:, loc] = sum_w feat8[w].T @ M_w
                acc = psA.tile([P, P], F32, tag="acc")
                for w in range(NW):
                    m16 = wp.tile([P, P], F16, tag=f"m16{w % 2}")
                    msl = m_ps[w // 4][:, (w % 4) * P:(w % 4 + 1) * P]
                    if w % 2 == 0:
                        nc.scalar.copy(m16[:], msl)
                    else:
                        nc.vector.tensor_copy(m16[:], msl)
                    nc.tensor.matmul(
                        acc[:], feat8(w), m16[:],
                        start=(w == 0), stop=(w == NW - 1),
                        skip_group_check=True)
                out_sb = wp.tile([P, P], F32, tag=f"out_sb{s}")
                nc.vector.tensor_copy(out_sb[:], acc[:])
                nc.sync.dma_start(out_d[:], out_sb[:])

    nc.compile()
    return nc


_PROGRAM = None


def _get_program():
    global _PROGRAM
    if _PROGRAM is None:
        _PROGRAM = _build_program()
    return _PROGRAM


def _pipe_arrays(key, arb, base):
    """key: bucketing key values (src for img pipe); arb: other endpoint.
    Returns ohkt, ohlt [P, EC] f16 (transposed one-hots), ohlo [P, EC]
    f16 (block-diagonal [e, lo] one-hot), loc8 [P, NBLK] f32."""
    kloc = key - base                 # 0..127
    w = arb >> 7                      # bucket
    lo = arb & 127
    slots = np.full(EC, -1, np.int64)  # slot -> edge index or -1
    fill = np.zeros(NW, np.int64)
    order = np.argsort(w, kind="stable")
    for ei in order:
        wb = w[ei]
        assert fill[wb] < BW, f"bucket overflow: {fill[wb]}"
        slots[wb * BW + fill[wb]] = ei
        fill[wb] += 1
    klocs = np.full(EC, -1, np.int64)
    los = np.full(EC, -1, np.int64)
    used = slots >= 0
    klocs[used] = kloc[slots[used]]
    los[used] = lo[slots[used]]
    rng = np.arange(P)
    ohkt = np.ascontiguousarray((klocs[None, :] == rng[:, None]).astype(np.float16))
    ohlt = np.ascontiguousarray((los[None, :] == rng[:, None]).astype(np.float16))
    ohlo = np.zeros((P, EC), np.float16)
    for b in range(NBLK):
        blk = los[b * P:(b + 1) * P]
        ohlo[:, b * P:(b + 1) * P] = blk[:, None] == rng[None, :]
    ohlo = np.ascontiguousarray(ohlo)
    loc8 = np.ascontiguousarray(klocs.astype(np.float32).reshape(NBLK, P).T)
    return ohkt, ohlt, ohlo, loc8


def _make_in_maps(img_features, text_features, src, tgt, W1, b1, w2, b2):
    img = np.asarray(img_features, np.float32)
    txt = np.asarray(text_features, np.float32)
    W1 = np.asarray(W1, np.float32)
    U = W1[:, :P] @ img                      # [H, IMG_DIM]
    V = W1[:, P:] @ txt                      # [H, TXT_DIM]
    # [lo, w, h] tables: T8[lo, w, :] = X[:, 128w + lo]
    V8 = np.ascontiguousarray(
        V.T.reshape(NW, P, P).transpose(1, 0, 2).reshape(P, NW * P)
    ).astype(np.float16)
    U8 = np.ascontiguousarray(
        U.T.reshape(NW, P, P).transpose(1, 0, 2).reshape(P, NW * P)
    ).astype(np.float16)
    txtT8 = np.ascontiguousarray(
        txt.T.reshape(NW, P, P).transpose(1, 0, 2).reshape(P, NW * P)
    ).astype(np.float16)
    imgT8 = np.ascontiguousarray(
        img.T.reshape(NW, P, P).transpose(1, 0, 2).reshape(P, NW * P)
    ).astype(np.float16)
    iota = np.tile(np.arange(P, dtype=np.float16)[None, :], (P, 1))
    ident = np.eye(P, dtype=np.float16)
    w2c = np.asarray(w2, np.float32).reshape(P, 1).astype(np.float16)
    b1c = np.asarray(b1, np.float32).reshape(P, 1)
    b2c = np.full((P, 1), np.float32(b2), dtype=np.float32)
    src = np.asarray(src).astype(np.int64)
    tgt = np.asarray(tgt).astype(np.int64)

    in_maps = []
    for c in range(NCORES):
        base = c * P
        UwinT = np.ascontiguousarray(U[:, base:base + P].T).astype(np.float16)
        VwinT = np.ascontiguousarray(V[:, base:base + P].T).astype(np.float16)
        p16 = np.empty((P, F16_FREE), np.float16)
        p16[:, OFF_WINT:OFF_WINT + P] = UwinT
        p16[:, OFF_WINT + P:OFF_WINT + 2 * P] = VwinT
        p16[:, OFF_TAB:OFF_TAB + NW * P] = V8
        p16[:, OFF_TAB + NW * P:OFF_TAB + 2 * NW * P] = U8
        p16[:, OFF_FEAT:OFF_FEAT + NW * P] = txtT8
        p16[:, OFF_FEAT + NW * P:OFF_FEAT + 2 * NW * P] = imgT8
        p16[:, OFF_W2:OFF_W2 + 1] = w2c
        p16[:, OFF_IOTA:OFF_IOTA + P] = iota
        p16[:, OFF_ID:OFF_ID + P] = ident
        p32 = np.empty((P, F32_FREE), np.float32)
        p32[:, P32_B1:P32_B1 + 1] = b1c
        p32[:, P32_B2:P32_B2 + 1] = b2c
        m = {"pack16": np.ascontiguousarray(p16)}
        for s, key, arb in (("i", src, tgt), ("t", tgt, src)):
            sel = (key >= base) & (key < base + P)
            ohkt, ohlt, ohlo, loc8 = _pipe_arrays(key[sel], arb[sel], base)
            m[f"{s}_ohkt"] = ohkt
            m[f"{s}_ohlt"] = ohlt
            m[f"{s}_ohlo"] = ohlo
            p32[:, (P32_LOC_I if s == "i" else P32_LOC_T):
                (P32_LOC_I if s == "i" else P32_LOC_T) + NBLK] = loc8
        m["pack32"] = np.ascontiguousarray(p32)
        in_maps.append(m)
    return in_maps


def _run(inputs, trace=False):
    from concourse.bass_utils import run_bass_kernel_spmd

    nc = _get_program()
    in_maps = _make_in_maps(**inputs)
    res = run_bass_kernel_spmd(
        nc, in_maps, core_ids=list(range(NCORES)), trace=trace
    )
    att_img = np.concatenate([r["out_img"] for r in res.results], axis=1)
    att_txt = np.concatenate([r["out_txt"] for r in res.results], axis=1)
    return (np.ascontiguousarray(att_img), np.ascontiguousarray(att_txt)), res


def kernel(**inputs):
    out, _ = _run(inputs, trace=False)
    return out


# revision 8
# speedup vs baseline: 2.1726x; 2.1726x over previous
"""Trainium2 Bass kernel for nn_CausalAttention (GNN message passing).

Math (reference):
    pairs[e] = [img[:, src[e]] ; text[:, tgt[e]]]          # B == H == 128
    a[e]     = sigmoid(w2 . relu(W1 @ pairs[e] + b1) + b2) # per-edge gate
    att_img[b, i] = sum_{e: src[e]=i} a[e] * text[b, tgt[e]]
    att_txt[b, t] = sum_{e: tgt[e]=t} a[e] * img[b, src[e]]

Architecture: output-column sharding, fully on-chip. Core c owns
att_img[:, Wc] and att_txt[:, Wc], Wc = [128c, 128c+128).
All tables are precomputed on host in f32 and shipped f16 (tolerance is
2e-2; f16 end-to-end error is ~1e-3). For the img pipe (txt symmetric):
  - edges with src in Wc, bucketed by w = tgt >> 7 (8 buckets of 5
    blocks of 128 edge slots; unused slots are dummies with -1 keys).
  - phase A (gate): h = relu(UwinT.T @ ohKT + V8[w].T @ ohLT + b1) via
    one-hot gather matmuls (f16, PSUM f32), relu on ACT into h16.
  - w2 dot: 10 matmuls with w2 [128,1] stationary over h16 chunks of
    512 -> z rows [10, 512]; sigmoid on ACT; 4 PE transposes put a into
    slot layout a_slot[p, col], col = 10*(b%4) + b//4 for block b.
  - phase B: per block, ohKa = (iota==loc)*a on DVE/GPSIMD (alternating)
    then M_w[lo, loc] += ohlo_b.T @ ohKa on PE (PSUM accum over 5 blocks).
  - tail: att[:, loc] = sum_w txtT8[w].T @ M_w (8 f16 matmuls).
"""

import sys

for _p in ("/opt/trn_rl_repo", "/root/.axon_site/_ro/trn_rl_repo"):
    if _p not in sys.path:
        sys.path.insert(0, _p)

import numpy as np

import concourse.bass as bass
import concourse.tile as tile
from concourse import bacc, mybir

P = 128
DIM = 1024
E = 32768
NCORES = 8
NW = 8            # hi buckets
BPW = 5           # blocks per bucket (capacity 640 vs mean 512)
NBLK = NW * BPW   # 40
EC = NBLK * P     # 5120 edge slots per pipeline
BW = BPW * P      # 640 edges per bucket
HC = EC // 2      # DMA half size

F32 = mybir.dt.float32
F16 = mybir.dt.float16

IS_EQ = mybir.AluOpType.is_equal
MULT = mybir.AluOpType.mult

# f16 const-pack free-dim layout (per partition)
OFF_WINT = 0                 # [0:128] UwinT (pipe i), [128:256] VwinT (pipe t)
OFF_TAB = 256                # V8 (pipe i) then U8 (pipe t), each 8*128
OFF_FEAT = OFF_TAB + 2048    # txtT8 (pipe i tail) then imgT8 (pipe t tail)
OFF_W2 = OFF_FEAT + 2048     # [.. : ..+1] w2 column
OFF_IOTA = OFF_W2 + 1        # [128, 128] iota along free dim
OFF_ID = OFF_IOTA + P        # [128, 128] identity
F16_FREE = OFF_ID + P        # 4609

# f32 pack: b1 | b2 | loc8_i | loc8_t
P32_B1 = 0
P32_B2 = 1
P32_LOC_I = 2
P32_LOC_T = 2 + NBLK
F32_FREE = 2 + 2 * NBLK


def _build_program():
    nc = bacc.Bacc(None, target_bir_lowering=False, debug=False)

    d16 = nc.dram_tensor("pack16", [P, F16_FREE], F16, kind="ExternalInput")
    d32 = nc.dram_tensor("pack32", [P, F32_FREE], F32, kind="ExternalInput")
    doh = {}
    for s in ("i", "t"):
        for k in ("ohkt", "ohlt", "ohlo"):
            doh[(s, k)] = nc.dram_tensor(
                f"{s}_{k}", [P, EC], F16, kind="ExternalInput")
    out_img = nc.dram_tensor("out_img", [P, P], F32, kind="ExternalOutput")
    out_txt = nc.dram_tensor("out_txt", [P, P], F32, kind="ExternalOutput")

    with tile.TileContext(nc) as tc:
        with (
            tc.tile_pool(name="const", bufs=1) as cp,
            tc.tile_pool(name="work", bufs=2) as wp,
            tc.tile_pool(name="psH", bufs=2, space="PSUM") as psH,
            tc.tile_pool(name="psW", bufs=1, space="PSUM") as psW,
            tc.tile_pool(name="psM", bufs=1, space="PSUM") as psM,
            tc.tile_pool(name="psA", bufs=1, space="PSUM") as psA,
        ):
            pack16 = cp.tile([P, F16_FREE], F16)
            pack32 = cp.tile([P, F32_FREE], F32)
            nc.sync.dma_start(pack16[:], d16[:])
            nc.sync.dma_start(pack32[:], d32[:])

            oh = {}
            for s in ("i", "t"):
                for k in ("ohkt", "ohlt", "ohlo"):
                    oh[(s, k)] = cp.tile([P, EC], F16, tag=f"{s}{k}",
                                         name=f"oh_{s}_{k}")
            # pipe-i masks first (they gate phase A), spread over 3 rings;
            # pipe-t masks queue behind them on the same rings.
            h0 = slice(0, HC)
            h1 = slice(HC, EC)
            nc.scalar.dma_start(oh[("i", "ohkt")][:, h0],
                                doh[("i", "ohkt")][:, h0])
            nc.sync.dma_start(oh[("i", "ohlt")][:, h0],
                              doh[("i", "ohlt")][:, h0])
            nc.gpsimd.dma_start(oh[("i", "ohkt")][:, h1],
                                doh[("i", "ohkt")][:, h1])
            nc.scalar.dma_start(oh[("i", "ohlt")][:, h1],
                                doh[("i", "ohlt")][:, h1])
            nc.sync.dma_start(oh[("i", "ohlo")][:], doh[("i", "ohlo")][:])
            nc.gpsimd.dma_start(oh[("t", "ohkt")][:], doh[("t", "ohkt")][:])
            nc.scalar.dma_start(oh[("t", "ohlt")][:], doh[("t", "ohlt")][:])
            nc.sync.dma_start(oh[("t", "ohlo")][:], doh[("t", "ohlo")][:])

            b1c = pack32[:, P32_B1:P32_B1 + 1]
            b2c = pack32[:, P32_B2:P32_B2 + 1]
            w2c = pack16[:, OFF_W2:OFF_W2 + 1]
            iota16 = pack16[:, OFF_IOTA:OFF_IOTA + P]
            id16 = pack16[:, OFF_ID:OFF_ID + P]

            for si, (s, out_d) in enumerate((("i", out_img), ("t", out_txt))):
                winT = pack16[:, OFF_WINT + si * P:OFF_WINT + (si + 1) * P]
                loc8 = pack32[:, (P32_LOC_I if s == "i" else P32_LOC_T):
                              (P32_LOC_I if s == "i" else P32_LOC_T) + NBLK]
                ohkt = oh[(s, "ohkt")]
                ohlt = oh[(s, "ohlt")]
                ohlo = oh[(s, "ohlo")]
                h16 = cp.tile([P, EC], F16, tag=f"h16{s}")
                a_slot = cp.tile([P, NBLK], F32, tag=f"a_slot{s}")

                def tab8(w):
                    o = OFF_TAB + si * 1024 + w * P
                    return pack16[:, o:o + P]

                def feat8(w):
                    o = OFF_FEAT + si * 1024 + w * P
                    return pack16[:, o:o + P]

                # ---- phase A: h = relu(winT.T@ohKT + tab8[w].T@ohLT + b1)
                for w in range(NW):
                    e0 = w * BW
                    h_ps = psH.tile([P, BW], F32, tag="h_ps")
                    for o, n in ((0, 512), (512, P)):
                        nc.tensor.matmul(
                            h_ps[:, o:o + n], winT, ohkt[:, e0 + o:e0 + o + n],
                            start=True, stop=False)
                    for o, n in ((0, 512), (512, P)):
                        nc.tensor.matmul(
                            h_ps[:, o:o + n], tab8(w), ohlt[:, e0 + o:e0 + o + n],
                            start=False, stop=True)
                    nc.scalar.activation(
                        h16[:, e0:e0 + BW], h_ps[:],
                        mybir.ActivationFunctionType.Relu, bias=b1c)

                # ---- w2 dot: per-block N=1 matmuls (h16 f16 stationary),
                # lands z directly in slot layout [p, block]
                a_ps = psW.tile([P, NBLK], F32, tag="a_ps")
                for b in range(NBLK):
                    nc.tensor.matmul(
                        a_ps[:, b:b + 1], h16[:, b * P:(b + 1) * P], w2c,
                        start=True, stop=True, skip_group_check=True)
                nc.scalar.activation(
                    a_slot[:], a_ps[:],
                    mybir.ActivationFunctionType.Sigmoid, bias=b2c)

                # ---- phase B: M_w[lo, loc] += ohlo_b.T @ ((iota==loc)*a)
                m_ps0 = psM.tile([P, 4 * P], F32, tag="m0")
                m_ps1 = psM.tile([P, 4 * P], F32, tag="m1")
                m_ps = [m_ps0, m_ps1]
                for b in range(NBLK):
                    w, j = b // BPW, b % BPW
                    oh_a = wp.tile([P, P], F16, tag=f"ohKa{b % 4}")
                    nc.vector.tensor_scalar(
                        out=oh_a[:], in0=iota16,
                        scalar1=loc8[:, b:b + 1], scalar2=a_slot[:, b:b + 1],
                        op0=IS_EQ, op1=MULT)
                    mslice = m_ps[w // 4][:, (w % 4) * P:(w % 4 + 1) * P]
                    nc.tensor.matmul(
                        mslice, ohlo[:, b * P:(b + 1) * P], oh_a[:],
                        start=(j == 0), stop=(j == BPW - 1),
                        skip_group_check=True)

                # ---- tail: att[:, loc] = sum_w feat8[w].T @ M_w
                acc = psA.tile([P, P], F32, tag="acc")
                for w in range(NW):
                    m16 = wp.tile([P, P], F16, tag=f"m16{w % 2}")
                    msl = m_ps[w // 4][:, (w % 4) * P:(w % 4 + 1) * P]
                    if w % 2 == 0:
                        nc.scalar.copy(m16[:], msl)
                    else:
                        nc.vector.tensor_copy(m16[:], msl)
                    nc.tensor.matmul(
                        acc[:], feat8(w), m16[:],
                        start=(w == 0), stop=(w == NW - 1),
                        skip_group_check=True)
                out_sb = wp.tile([P, P], F32, tag=f"out_sb{s}")
                nc.vector.tensor_copy(out_sb[:], acc[:])
                nc.sync.dma_start(out_d[:], out_sb[:])

    nc.compile()
    return nc


_PROGRAM = None


def _get_program():
    global _PROGRAM
    if _PROGRAM is None:
        _PROGRAM = _build_program()
    return _PROGRAM


def _pipe_arrays(key, arb, base):
    """key: bucketing key values (src for img pipe); arb: other endpoint.
    Returns ohkt, ohlt [P, EC] f16 (transposed one-hots), ohlo [P, EC]
    f16 (block-diagonal [e, lo] one-hot), loc8 [P, NBLK] f32."""
    kloc = key - base                 # 0..127
    w = arb >> 7                      # bucket
    lo = arb & 127
    slots = np.full(EC, -1, np.int64)  # slot -> edge index or -1
    fill = np.zeros(NW, np.int64)
    order = np.argsort(w, kind="stable")
    for ei in order:
        wb = w[ei]
        assert fill[wb] < BW, f"bucket overflow: {fill[wb]}"
        slots[wb * BW + fill[wb]] = ei
        fill[wb] += 1
    klocs = np.full(EC, -1, np.int64)
    los = np.full(EC, -1, np.int64)
    used = slots >= 0
    klocs[used] = kloc[slots[used]]
    los[used] = lo[slots[used]]
    rng = np.arange(P)
    ohkt = np.ascontiguousarray((klocs[None, :] == rng[:, None]).astype(np.float16))
    ohlt = np.ascontiguousarray((los[None, :] == rng[:, None]).astype(np.float16))
    ohlo = np.zeros((P, EC), np.float16)
    for b in range(NBLK):
        blk = los[b * P:(b + 1) * P]
        ohlo[:, b * P:(b + 1) * P] = blk[:, None] == rng[None, :]
    ohlo = np.ascontiguousarray(ohlo)
    loc8 = np.ascontiguousarray(klocs.astype(np.float32).reshape(NBLK, P).T)
    return ohkt, ohlt, ohlo, loc8


def _make_in_maps(img_features, text_features, src, tgt, W1, b1, w2, b2):
    img = np.asarray(img_features, np.float32)
    txt = np.asarray(text_features, np.float32)
    W1 = np.asarray(W1, np.float32)
    U = W1[:, :P] @ img                      # [H, IMG_DIM]
    V = W1[:, P:] @ txt                      # [H, TXT_DIM]
    # [lo, w, h] tables: T8[lo, w, :] = X[:, 128w + lo]
    V8 = np.ascontiguousarray(
        V.T.reshape(NW, P, P).transpose(1, 0, 2).reshape(P, NW * P)
    ).astype(np.float16)
    U8 = np.ascontiguousarray(
        U.T.reshape(NW, P, P).transpose(1, 0, 2).reshape(P, NW * P)
    ).astype(np.float16)
    txtT8 = np.ascontiguousarray(
        txt.T.reshape(NW, P, P).transpose(1, 0, 2).reshape(P, NW * P)
    ).astype(np.float16)
    imgT8 = np.ascontiguousarray(
        img.T.reshape(NW, P, P).transpose(1, 0, 2).reshape(P, NW * P)
    ).astype(np.float16)
    iota = np.tile(np.arange(P, dtype=np.float16)[None, :], (P, 1))
    ident = np.eye(P, dtype=np.float16)
    w2c = np.asarray(w2, np.float32).reshape(P, 1).astype(np.float16)
    b1c = np.asarray(b1, np.float32).reshape(P, 1)
    b2c = np.full((P, 1), np.float32(b2), dtype=np.float32)
    src = np.asarray(src).astype(np.int64)
    tgt = np.asarray(tgt).astype(np.int64)

    in_maps = []
    for c in range(NCORES):
        base = c * P
        UwinT = np.ascontiguousarray(U[:, base:base + P].T).astype(np.float16)
        VwinT = np.ascontiguousarray(V[:, base:base + P].T).astype(np.float16)
        p16 = np.empty((P, F16_FREE), np.float16)
        p16[:, OFF_WINT:OFF_WINT + P] = UwinT
        p16[:, OFF_WINT + P:OFF_WINT + 2 * P] = VwinT
        p16[:, OFF_TAB:OFF_TAB + NW * P] = V8
        p16[:, OFF_TAB + NW * P:OFF_TAB + 2 * NW * P] = U8
        p16[:, OFF_FEAT:OFF_FEAT + NW * P] = txtT8
        p16[:, OFF_FEAT + NW * P:OFF_FEAT + 2 * NW * P] = imgT8
        p16[:, OFF_W2:OFF_W2 + 1] = w2c
        p16[:, OFF_IOTA:OFF_IOTA + P] = iota
        p16[:, OFF_ID:OFF_ID + P] = ident
        p32 = np.empty((P, F32_FREE), np.float32)
        p32[:, P32_B1:P32_B1 + 1] = b1c
        p32[:, P32_B2:P32_B2 + 1] = b2c
        m = {"pack16": np.ascontiguousarray(p16)}
        for s, key, arb in (("i", src, tgt), ("t", tgt, src)):
            sel = (key >= base) & (key < base + P)
            ohkt, ohlt, ohlo, loc8 = _pipe_arrays(key[sel], arb[sel], base)
            m[f"{s}_ohkt"] = ohkt
            m[f"{s}_ohlt"] = ohlt
            m[f"{s}_ohlo"] = ohlo
            p32[:, (P32_LOC_I if s == "i" else P32_LOC_T):
                (P32_LOC_I if s == "i" else P32_LOC_T) + NBLK] = loc8
        m["pack32"] = np.ascontiguousarray(p32)
        in_maps.append(m)
    return in_maps


def _run(inputs, trace=False):
    from concourse.bass_utils import run_bass_kernel_spmd

    nc = _get_program()
    in_maps = _make_in_maps(**inputs)
    res = run_bass_kernel_spmd(
        nc, in_maps, core_ids=list(range(NCORES)), trace=trace
    )
    att_img = np.concatenate([r["out_img"] for r in res.results], axis=1)
    att_txt = np.concatenate([r["out_txt"] for r in res.results], axis=1)
    return (np.ascontiguousarray(att_img), np.ascontiguousarray(att_txt)), res


def kernel(**inputs):
    out, _ = _run(inputs, trace=False)
    return out
